# revision 13
# baseline (speedup 1.0000x reference)
"""Trainium2 Bass kernel for nn_CrossNonLocalBlock (sparse_attention).

Strategy
--------
Pure data parallelism over batch B=16 across 8 NeuronCores (2 items/core).
Per item, the [N,N] attention matrix is never materialized: with
t' = t*D, p' = p*D (D = diag(dinv)),

    f      = 0.5 * D (t^T p + p^T t) D            (rank <= 256)
    agg    = G f = 0.5 * [(G t'^T) p' + (G p'^T) t']
    d[n]   = 0.5 * (t^T p_sum + p^T t_sum)[n]      (row sums via matvec)

Stage A computes M1T = (t' G^T)^T and M2T = (p' G^T)^T ([IC,IC] blocks,
contraction over N), stage B computes agg = M1T.T @ p + M2T.T @ t
(contraction over IC) with the right-hand D and the 0.5 folded into the
PSUM eviction (tensor * dinv_broadcast * 0.5).

BatchNorm biases cancel (BN is shift invariant) so out-conv biases are
dropped; bwx is folded into bout on the host (bout' = bout + Wout@bwx).
Only BN batch statistics cross cores: one 4KB AllReduce of
(sum, sumsq) x 2 layers.

All matmul operands are bf16 (full-rate on PE); accumulation, BN stats,
dinv pipeline, residual and output are fp32.
"""

import os
import sys

for _p in ("/opt/trn_rl_repo",):
    if _p not in sys.path and os.path.isdir(_p):
        sys.path.insert(0, _p)

import numpy as np
import ml_dtypes

import concourse.bass as bass
import concourse.bacc as bacc
import concourse.mybir as mybir
from concourse import tile

BF16 = mybir.dt.bfloat16
F32 = mybir.dt.float32
AF = mybir.ActivationFunctionType
ALU = mybir.AluOpType

B, C, IC, H, W = 16, 256, 128, 32, 32
N = H * W                      # 1024
NCORES = 8
ITEMS = B // NCORES            # 2 items per core
CNT = float(B * N)             # global batchnorm count
EPS = 1e-5

_cache = {}


def _build_nc():
    """Build the per-core Bass program (identical on all 8 cores)."""
    nc = bacc.Bacc("TRN2", target_bir_lowering=False, debug=False,
                   num_devices=NCORES)

    # ---- DRAM I/O ------------------------------------------------------
    # inputs in core layout: [item, c_chunk, 128, N]
    xf_d = nc.dram_tensor("xf", [ITEMS, 2, 128, N], F32, kind="ExternalInput")
    xb_d = nc.dram_tensor("xb", [ITEMS, 2, 128, N], BF16, kind="ExternalInput")
    ob_d = nc.dram_tensor("obb", [ITEMS, 2, 128, N], BF16, kind="ExternalInput")
    od_d = nc.dram_tensor("odb", [ITEMS, 2, 128, N], BF16, kind="ExternalInput")
    # weights
    wn_d = {s: nc.dram_tensor(f"wn_{s}", [2, 128, 3 * IC], BF16,
                              kind="ExternalInput") for s in "xbd"}
    wc_d = nc.dram_tensor("wc", [2, 128, 2 * IC], BF16, kind="ExternalInput")
    gb_d = nc.dram_tensor("gbias", [128, 3, IC], F32, kind="ExternalInput")
    wo_d = nc.dram_tensor("wo", [128, 5, C], BF16, kind="ExternalInput")
    wout_d = nc.dram_tensor("wout", [2, 128, C], BF16, kind="ExternalInput")
    bout_d = nc.dram_tensor("bout2", [128, 2], F32, kind="ExternalInput")
    gamb_d = nc.dram_tensor("gamb", [128, 2, 4], F32, kind="ExternalInput")
    out_d = nc.dram_tensor("out", [ITEMS, 2, 128, N], F32, kind="ExternalOutput")
    # scratch + collective buffers
    dscr = nc.dram_tensor("dscratch", [ITEMS, 3, 1, N], F32)
    ar_in = nc.dram_tensor("ar_in", [128, 8], F32)
    ar_out = nc.dram_tensor("ar_out", [128, 8], F32, addr_space="Shared")

    SRCS = ("x", "b", "d")
    TGTS = {"x": ("x",), "b": ("b", "x"), "d": ("d", "x")}

    from contextlib import ExitStack
    with tile.TileContext(nc) as tc, ExitStack() as es:
        wp = es.enter_context(tc.tile_pool(name="wts", bufs=1))
        io = es.enter_context(tc.tile_pool(name="io", bufs=2))
        fe = es.enter_context(tc.tile_pool(name="feat", bufs=4))
        g3 = es.enter_context(tc.tile_pool(name="gpool", bufs=3))
        ag = es.enter_context(tc.tile_pool(name="aggp", bufs=3))
        rt = es.enter_context(tc.tile_pool(name="ret", bufs=2))
        sm = es.enter_context(tc.tile_pool(name="small", bufs=2))
        ps_s = es.enter_context(tc.tile_pool(name="psA", bufs=2, space="PSUM"))
        ps_n = es.enter_context(tc.tile_pool(name="psN", bufs=2, space="PSUM"))
        ps_b = es.enter_context(tc.tile_pool(name="psB", bufs=2, space="PSUM"))

        # ---- load weights ---------------------------------------------
        wn = {}
        for s in SRCS:
            wn[s] = wp.tile([128, 2, 3 * IC], BF16, tag=f"wn{s}", name=f"wn{s}")
            nc.sync.dma_start(wn[s][:], wn_d[s].rearrange("k p m -> p k m"))
        wc = wp.tile([128, 2, 2 * IC], BF16, tag="wc")
        nc.sync.dma_start(wc[:], wc_d.rearrange("k p m -> p k m"))
        gbias = wp.tile([128, 3, IC], F32, tag="gb")
        nc.sync.dma_start(gbias[:], gb_d[:])
        wo = wp.tile([128, 5, C], BF16, tag="wo")
        nc.sync.dma_start(wo[:], wo_d[:])
        wout = wp.tile([128, 2, C], BF16, tag="wout")
        nc.sync.dma_start(wout[:], wout_d.rearrange("k p m -> p k m"))
        bout2 = wp.tile([128, 2], F32, tag="bout")
        nc.sync.dma_start(bout2[:], bout_d[:])
        gamb = wp.tile([128, 2, 4], F32, tag="gamb")
        nc.sync.dma_start(gamb[:], gamb_d[:])

        stats_items = []
        sums_items = []       # (sum1, sum2, xself, Xf) per item

        for it in range(ITEMS):
            # ---- load inputs ------------------------------------------
            Xf = io.tile([128, 2, N], F32, tag="xf")
            for mc in range(2):
                nc.sync.dma_start(Xf[:, mc, :], xf_d[it, mc])
            Xin = {}
            for s, dram in (("x", xb_d), ("b", ob_d), ("d", od_d)):
                t = io.tile([128, 2, N], BF16, tag=f"in_{s}", name=f"in_{s}")
                for mc in range(2):
                    nc.sync.dma_start(t[:, mc, :], dram[it, mc])
                Xin[s] = t

            feats = {}
            for si, s in enumerate(SRCS):
                Xs = Xin[s]
                # ---- c-layout convs t, p (relu + row-sum fused) -------
                cps = {}
                csum = {}
                for wi, name in enumerate(("t", "p")):
                    ps = ps_b.tile([128, N], F32, tag="big")
                    for nn in range(2):
                        for kc in range(2):
                            nc.tensor.matmul(
                                ps[:, nn * 512:(nn + 1) * 512],
                                wc[:, kc, wi * IC:(wi + 1) * IC],
                                Xs[:, kc, nn * 512:(nn + 1) * 512],
                                start=(kc == 0), stop=(kc == 1))
                    sb = fe.tile([128, N], BF16, tag=f"c{name}", bufs=3)
                    ssum = sm.tile([128, 1], F32, tag=f"s{name}", )
                    nc.scalar.activation(sb[:], ps[:], AF.Relu, accum_out=ssum[:])
                    cps[name] = sb
                    csum[name] = ssum
                Tc, Pc = cps["t"], cps["p"]

                # ---- d = 0.5*(p_sum^T t + t_sum^T p) ----------------
                psb = sm.tile([128, 1], BF16, tag="psb")
                tsb = sm.tile([128, 1], BF16, tag="tsb")
                nc.vector.tensor_scalar_mul(psb[:], csum["p"][:], 0.5)
                nc.vector.tensor_scalar_mul(tsb[:], csum["t"][:], 0.5)
                d_lo = ps_n.tile([1, 512], F32, tag="nconv")
                d_hi = ps_s.tile([1, 512], F32, tag="stageA")
                dps = [d_lo, d_hi]
                for half in range(2):
                    sl = slice(half * 512, (half + 1) * 512)
                    nc.tensor.matmul(dps[half][:], psb[:], Tc[:, sl],
                                     start=True, stop=False)
                    nc.tensor.matmul(dps[half][:], tsb[:], Pc[:, sl],
                                     start=False, stop=True)
                dsb = sm.tile([1, N], F32, tag="dsb")
                for half in range(2):
                    nc.scalar.copy(dsb[:, half * 512:(half + 1) * 512],
                                   dps[half][:])
                # ---- dinv = where(d>0, rsqrt(d), 0), in place ---------
                msk = sm.tile([1, N], F32, tag="msk")
                nc.vector.tensor_scalar(msk[:], dsb[:], 0.0, None, ALU.is_gt)
                nc.vector.scalar_tensor_tensor(dsb[:], dsb[:], 1.0, msk[:],
                                               ALU.add, ALU.subtract)
                nc.vector.reciprocal(dsb[:], dsb[:])
                nc.scalar.sqrt(dsb[:], dsb[:])
                dv = dsb
                nc.vector.tensor_mul(dv[:], dsb[:], msk[:])
                # broadcast across partitions (for agg eviction scaling)
                dbc = g3.tile([128, N], F32, tag="dbc")
                nc.gpsimd.partition_broadcast(dbc[:], dv[:])
                # [128, 8] chunk-scalar layout via DRAM round trip
                nc.sync.dma_start(dscr[it, si], dv[:])
                dch = sm.tile([128, 8], F32, tag="dch")
                nc.sync.dma_start(
                    dch[:], dscr[it, si, 0].rearrange("(k p) -> p k", p=128))

                # ---- n-layout convs [Tn'|Pn'|Gn] ----------------------
                Tn = fe.tile([128, 8, IC], BF16, tag="tn", bufs=3)
                Pn = fe.tile([128, 8, IC], BF16, tag="pn", bufs=3)
                Gn = g3.tile([128, 8, IC], BF16, tag="gn")
                for nk in range(8):
                    nps = ps_n.tile([128, 3 * IC], F32, tag="nconv")
                    for kc in range(2):
                        nc.tensor.matmul(
                            nps[:], Xs[:, kc, nk * 128:(nk + 1) * 128],
                            wn[s][:, kc, :], start=(kc == 0), stop=(kc == 1))
                    sc = dch[:, nk:nk + 1]
                    nc.scalar.activation(Tn[:, nk, :], nps[:, 0:IC],
                                         AF.Relu, scale=sc)
                    nc.scalar.activation(Pn[:, nk, :], nps[:, IC:2 * IC],
                                         AF.Relu, scale=sc)
                    nc.vector.tensor_add(Gn[:, nk, :], nps[:, 2 * IC:3 * IC],
                                         gbias[:, si, :])
                feats[s] = dict(Tc=Tc, Pc=Pc, Tn=Tn, Pn=Pn, Gn=Gn, dbc=dbc)

            # ---- stage A: M1T/M2T blocks ------------------------------
            ablk = {}
            for s in SRCS:
                tgts = TGTS[s]
                aps = ps_s.tile([128, 512], F32, tag="stageA")
                width = 256 * len(tgts)
                first = True
                for nk in range(8):
                    for ti_, tp in enumerate((feats[s]["Tn"], feats[s]["Pn"])):
                        for gi, tgt in enumerate(tgts):
                            last = (nk == 7 and ti_ == 1 and gi == len(tgts) - 1)
                            nc.tensor.matmul(
                                aps[:, gi * 256 + ti_ * 128:
                                       gi * 256 + (ti_ + 1) * 128],
                                tp[:, nk, :], feats[tgt]["Gn"][:, nk, :],
                                start=first, stop=last)
                            first = False
                ab = ag.tile([128, 512], BF16, tag="ablk")
                nc.vector.tensor_copy(ab[:, 0:width], aps[:, 0:width])
                for gi, tgt in enumerate(tgts):
                    ablk[(s, tgt)] = ab[:, gi * 256:(gi + 1) * 256]

            # ---- stage B + out-convs ----------------------------------
            def stage_b(src, tgt):
                f = feats[src]
                blk = ablk[(src, tgt)]
                agps = ps_b.tile([128, N], F32, tag="big")
                for nn in range(2):
                    sl = slice(nn * 512, (nn + 1) * 512)
                    nc.tensor.matmul(agps[:, sl], blk[:, 0:128], f["Pc"][:, sl],
                                     start=True, stop=False)
                    nc.tensor.matmul(agps[:, sl], blk[:, 128:256], f["Tc"][:, sl],
                                     start=False, stop=True)
                a = ag.tile([128, N], BF16, tag="agg")
                nc.vector.scalar_tensor_tensor(a[:], agps[:], 0.5, f["dbc"][:],
                                               ALU.mult, ALU.mult)
                return a

            stats = sm.tile([128, 8], F32, tag=f"stats{it}")
            sum1 = rt.tile([128, 2, N], F32, tag="sum1")
            sum2 = rt.tile([128, 2, N], F32, tag="sum2")
            xself = rt.tile([128, 2, N], F32, tag="xself")

            # (dest, (wo index, src, tgt) convs, bn layer or None)
            plans = [
                (sum1, ((2, "d", "d"), (3, "b", "x")), 0),
                (sum2, ((1, "b", "b"), (4, "d", "x")), 1),
                (xself, ((0, "x", "x"),), None),
            ]
            for dest, convs, layer in plans:
                aggs = [(wi, stage_b(s_, t_)) for wi, s_, t_ in convs]
                for mc in range(2):
                    ps = ps_b.tile([128, N], F32, tag="big")
                    for ci, (wi, a) in enumerate(aggs):
                        for nn in range(2):
                            sl = slice(nn * 512, (nn + 1) * 512)
                            nc.tensor.matmul(
                                ps[:, sl], wo[:, wi, mc * 128:(mc + 1) * 128],
                                a[:, sl], start=(ci == 0), stop=(ci == len(aggs) - 1))
                    if layer is None:
                        nc.scalar.copy(dest[:, mc, :], ps[:])
                    else:
                        nc.scalar.activation(
                            dest[:, mc, :], ps[:], AF.Copy,
                            accum_out=stats[:, layer * 4 + mc:layer * 4 + mc + 1])
                        junk = fe.tile([128, N], BF16, tag="junk", bufs=2)
                        nc.scalar.activation(
                            junk[:], dest[:, mc, :], AF.Square,
                            accum_out=stats[:, layer * 4 + 2 + mc:
                                            layer * 4 + 3 + mc])
            stats_items.append(stats)
            sums_items.append((sum1, sum2, xself, Xf))

        # ---- AllReduce of BN statistics -------------------------------
        stot = sm.tile([128, 8], F32, tag="stot")
        nc.vector.tensor_add(stot[:], stats_items[0][:], stats_items[1][:])
        nc.sync.dma_start(ar_in[:], stot[:])
        nc.gpsimd.collective_compute(
            "AllReduce", ALU.add, replica_groups=[list(range(NCORES))],
            ins=[ar_in.ap()], outs=[ar_out.ap()])
        sg = sm.tile([128, 8], F32, tag="sg")
        nc.sync.dma_start(sg[:], ar_out[:])

        # ---- BN coefficients A, Bc per layer/chunk --------------------
        # gamb layout: [:, mc, (g1, b1, g2, b2)]
        A = sm.tile([128, 2, 2], F32, tag="A")     # [mc, layer]
        Bc = sm.tile([128, 2], F32, tag="Bc")      # combined B1+B2 per mc
        for mc in range(2):
            bparts = []
            for layer in range(2):
                s_ap = sg[:, layer * 4 + mc:layer * 4 + mc + 1]
                q_ap = sg[:, layer * 4 + 2 + mc:layer * 4 + 3 + mc]
                mu = sm.tile([128, 1], F32, tag="mu")
                nc.vector.tensor_scalar_mul(mu[:], s_ap, 1.0 / CNT)
                msq = sm.tile([128, 1], F32, tag="msq")
                nc.vector.tensor_scalar_mul(msq[:], q_ap, 1.0 / CNT)
                tmp = sm.tile([128, 4], F32, tag="tmp")
                nc.vector.tensor_mul(tmp[:, 0:1], mu[:], mu[:])
                # var + eps = (msq + eps) - mu^2
                nc.vector.scalar_tensor_tensor(tmp[:, 1:2], msq[:], EPS,
                                               tmp[:, 0:1], ALU.add,
                                               ALU.subtract)
                nc.scalar.activation(tmp[:, 2:3], tmp[:, 1:2], AF.Sqrt)
                nc.vector.reciprocal(tmp[:, 3:4], tmp[:, 2:3])
                a_ap = A[:, mc, layer:layer + 1]
                g_ap = gamb[:, mc, 2 * layer:2 * layer + 1]
                b_ap = gamb[:, mc, 2 * layer + 1:2 * layer + 2]
                nc.vector.tensor_mul(a_ap, g_ap, tmp[:, 3:4])
                bpart = sm.tile([128, 1], F32, tag=f"bp{layer}")
                nc.vector.tensor_mul(bpart[:], mu[:], a_ap)
                nc.vector.tensor_sub(bpart[:], b_ap, bpart[:])
                bparts.append(bpart)
            nc.vector.tensor_add(Bc[:, mc:mc + 1], bparts[0][:], bparts[1][:])

        # ---- BN apply + Wout + residual per item ----------------------
        for it in range(ITEMS):
            sum1, sum2, xself, Xf = sums_items[it]
            r3 = io.tile([128, 2, N], BF16, tag="r3")
            for mc in range(2):
                u = fe.tile([128, N], F32, tag="u", bufs=2)
                nc.gpsimd.tensor_scalar(u[:], sum1[:, mc, :], A[:, mc, 0:1],
                                        None, ALU.mult)
                v = fe.tile([128, N], F32, tag="v", bufs=2)
                nc.vector.scalar_tensor_tensor(v[:], sum2[:, mc, :],
                                               A[:, mc, 1:2], u[:],
                                               ALU.mult, ALU.add)
                nc.vector.scalar_tensor_tensor(r3[:, mc, :], v[:],
                                               Bc[:, mc:mc + 1],
                                               xself[:, mc, :],
                                               ALU.add, ALU.add)
            for mc in range(2):
                osb = io.tile([128, N], F32, tag="osb")
                ps = ps_b.tile([128, N], F32, tag="big")
                for kc in range(2):
                    for nn in range(2):
                        sl = slice(nn * 512, (nn + 1) * 512)
                        nc.tensor.matmul(ps[:, sl],
                                         wout[:, kc, mc * 128:(mc + 1) * 128],
                                         r3[:, kc, sl],
                                         start=(kc == 0), stop=(kc == 1))
                nc.vector.scalar_tensor_tensor(osb[:], ps[:],
                                               bout2[:, mc:mc + 1],
                                               Xf[:, mc, :], ALU.add, ALU.add)
                nc.sync.dma_start(out_d[it, mc], osb[:])

    nc.compile()
    return nc


def _prep_maps(inputs):
    """Host-side sharding + weight prep -> per-core input maps."""
    bf = ml_dtypes.bfloat16
    f32 = np.float32

    def corely(a):
        # [B, C, H, W] -> per-core [ITEMS, 2, 128, N]
        a = np.ascontiguousarray(a.reshape(B, C, N)).reshape(
            NCORES, ITEMS, 2, 128, N)
        return a

    xs = corely(np.asarray(inputs["x"], f32))
    obs = corely(np.asarray(inputs["ob"], f32))
    ods = corely(np.asarray(inputs["od"], f32))

    wn = {}
    for s, wg, bg in (("x", "Wgx", "bgx"), ("b", "Wgb", "bgb"),
                      ("d", "Wgd", "bgd")):
        blk = np.concatenate([np.asarray(inputs["Wt"]).T,
                              np.asarray(inputs["Wp"]).T,
                              np.asarray(inputs[wg]).T], axis=1)  # [C, 384]
        wn[s] = np.ascontiguousarray(blk.reshape(2, 128, 3 * IC)).astype(bf)
    wc = np.concatenate([np.asarray(inputs["Wt"]).T,
                         np.asarray(inputs["Wp"]).T], axis=1)
    wc = np.ascontiguousarray(wc.reshape(2, 128, 2 * IC)).astype(bf)
    gbias = np.stack([np.tile(np.asarray(inputs[b], f32), (128, 1))
                      for b in ("bgx", "bgb", "bgd")], axis=1)  # [128, 3, IC]
    wo = np.stack([np.asarray(inputs[k], f32).T.astype(bf)
                   for k in ("Wwx", "Wwb", "Wwd", "Wwxb", "Wwxd")],
                  axis=1)  # [IC=128, 5, C]
    wout = np.ascontiguousarray(
        np.asarray(inputs["Wout"], f32).T.reshape(2, 128, C)).astype(bf)
    bout2 = (np.asarray(inputs["bout"], f32)
             + np.asarray(inputs["Wout"], f32) @ np.asarray(inputs["bwx"], f32))
    bout2 = np.ascontiguousarray(bout2.reshape(2, 128).T)  # [128, 2]
    gamb = np.stack([np.asarray(inputs[k], f32).reshape(2, 128).T
                     for k in ("gamma1", "beta1", "gamma2", "beta2")],
                    axis=2)  # [128, 2, 4]

    maps = []
    for c in range(NCORES):
        m = {
            "xf": np.ascontiguousarray(xs[c]),
            "xb": np.ascontiguousarray(xs[c]).astype(bf),
            "obb": np.ascontiguousarray(obs[c]).astype(bf),
            "odb": np.ascontiguousarray(ods[c]).astype(bf),
            "wc": wc, "gbias": np.ascontiguousarray(gbias),
            "wo": np.ascontiguousarray(wo.astype(bf)), "wout": wout,
            "bout2": np.ascontiguousarray(bout2),
            "gamb": np.ascontiguousarray(gamb),
        }
        for s in "xbd":
            m[f"wn_{s}"] = wn[s]
        maps.append(m)
    return maps


def _get_runner():
    """Build (once) a cached jitted 8-core runner modeled on
    bass2jax.run_bass_via_pjrt, so repeat calls don't re-jit."""
    if "runner" in _cache:
        return _cache["runner"]

    import jax
    from jax.experimental.shard_map import shard_map
    from jax.sharding import Mesh, PartitionSpec
    from concourse import bass2jax

    nc = _cache.get("nc")
    if nc is None:
        nc = _build_nc()
        _cache["nc"] = nc

    bass2jax.install_neuronx_cc_hook()

    partition_name = (nc.partition_id_tensor.name
                      if nc.partition_id_tensor else None)
    in_names, out_names, out_avals, zero_outs = [], [], [], []
    for alloc in nc.m.functions[0].allocations:
        if not isinstance(alloc, mybir.MemoryLocationSet):
            continue
        name = alloc.memorylocations[0].name
        if alloc.kind == "ExternalInput":
            if name != partition_name:
                in_names.append(name)
        elif alloc.kind == "ExternalOutput":
            out_names.append(name)
            shape = tuple(alloc.tensor_shape)
            dtype = mybir.dt.np(alloc.dtype)
            out_avals.append(jax.core.ShapedArray(shape, dtype))
            zero_outs.append(np.zeros(shape, dtype))
    n_params = len(in_names)
    n_outs = len(out_avals)
    all_names = in_names + out_names
    if partition_name is not None:
        all_names = all_names + [partition_name]

    def _body(*args):
        operands = list(args)
        if partition_name is not None:
            operands.append(bass2jax.partition_id_tensor())
        outs = bass2jax._bass_exec_p.bind(
            *operands,
            out_avals=tuple(out_avals),
            in_names=tuple(all_names),
            out_names=tuple(out_names),
            lowering_input_output_aliases=(),
            sim_require_finite=True,
            sim_require_nnan=True,
            nc=nc,
        )
        return tuple(outs)

    devices = jax.devices()[:NCORES]
    mesh = Mesh(np.asarray(devices), ("core",))
    donate = tuple(range(n_params, n_params + n_outs))
    sharded = jax.jit(
        shard_map(_body, mesh=mesh,
                  in_specs=(PartitionSpec("core"),) * (n_params + n_outs),
                  out_specs=(PartitionSpec("core"),) * n_outs,
                  check_rep=False),
        donate_argnums=donate, keep_unused=True)

    def run(in_maps):
        concat_in = [np.concatenate([np.asarray(m[name]) for m in in_maps],
                                    axis=0) for name in in_names]
        concat_zero = [np.zeros((NCORES * z.shape[0], *z.shape[1:]), z.dtype)
                       for z in zero_outs]
        out_arrs = sharded(*concat_in, *concat_zero)
        out_arrs = [np.asarray(a) for a in out_arrs]
        return [{name: out_arrs[i].reshape(NCORES, *out_avals[i].shape)[c]
                 for i, name in enumerate(out_names)}
                for c in range(NCORES)]

    _cache["runner"] = run
    return run


def kernel(**inputs):
    run = _get_runner()
    maps = _prep_maps(inputs)
    results = run(maps)
    out = np.stack([r["out"] for r in results], axis=0)  # [8, ITEMS, 2, 128, N]
    return np.ascontiguousarray(out.reshape(B, C, H, W))


if __name__ == "__main__":
    rng = np.random.default_rng(0)
    fake = {}
    fake["x"] = rng.standard_normal((B, C, H, W), dtype=np.float32)
    fake["ob"] = rng.standard_normal((B, C, H, W), dtype=np.float32)
    fake["od"] = rng.standard_normal((B, C, H, W), dtype=np.float32)
    for k, shp in (("Wt", (IC, C)), ("Wp", (IC, C)), ("Wgx", (IC, C)),
                   ("Wgb", (IC, C)), ("Wgd", (IC, C)), ("Wwx", (C, IC)),
                   ("Wwb", (C, IC)), ("Wwd", (C, IC)), ("Wwxb", (C, IC)),
                   ("Wwxd", (C, IC)), ("Wout", (C, C))):
        fake[k] = (rng.standard_normal(shp, dtype=np.float32) * 0.05)
    for k in ("bgx", "bgb", "bgd", "bwx", "bwb", "bwd", "bwxb", "bwxd", "bout"):
        fake[k] = rng.standard_normal(
            (IC,) if k.startswith("bg") else (C,), dtype=np.float32) * 0.05
    fake["gamma1"] = np.ones(C, np.float32)
    fake["beta1"] = np.zeros(C, np.float32)
    fake["gamma2"] = np.ones(C, np.float32)
    fake["beta2"] = np.zeros(C, np.float32)
    out = kernel(**fake)
    print("kernel ran:", out.shape, out.dtype, float(np.abs(out).mean()))
